# revision 56
# baseline (speedup 1.0000x reference)
"""Trainium2 Bass kernel for LocalDenseSynthesizerAttention (band C=63, H=4 heads).

Sharding: 8192 tokens (B=2 x T=4096 flattened) split contiguously across 8
cores (1024 tokens each).  Each core runs an identical program on its own
slice; batch-edge band masking and value halo padding are handled host-side
via per-core input data (masks / zero-padded valueT), so the program is
uniform SPMD.

Layouts on device are feature-major ("transposed"): activations live as
(feat, token) so every projection is a plain lhsT.T @ rhs matmul.  The band
scatter runs fully on-chip via the GPSIMD local_scatter instruction
(dst[:]=0; dst[:, idx]=data per partition), turning compact scores
(token, offset) into a banded matrix in SBUF, which is then PE-transposed
into (window_row, token) orientation for the band matmuls.  Copies and
softmax ops are distributed across Act/DVE/Pool to balance engine load.
"""

import numpy as np
import ml_dtypes

import concourse.bass as bass
import concourse.bacc as bacc
import concourse.mybir as mybir
import concourse.tile as tile
from concourse.ap import AP
from concourse import bass_utils

BF16 = mybir.dt.bfloat16
FP32 = mybir.dt.float32
NP_BF16 = ml_dtypes.bfloat16

B, T, NF = 2, 4096, 256
H, C, DK = 4, 63, 64
HALF = (C - 1) // 2  # 31
N_CORES = 8
TPC = (B * T) // N_CORES  # 1024 tokens per core
N_TILES = TPC // 128  # 8
VPAD = 1152  # parked value rows: tokens [-31, 1121) relative to core start
SW = 256  # per-head width of the transposed S^T buffer (2 chunks of 128)
SWS = 192  # per-head stride of the banded score buffer (window rows 0..189+pad)
SALL_W = 768  # banded buffer width (4 heads x 192)
DMA_T_TILES = ()  # tiles whose transposes run on the DMA xbar instead of PE
MUL_ENGINE = "vector"  # engine for the softmax normalize multiply
SPLIT_IN_DMA = True  # split wpack/vt input DMAs by first use
VPROJ_EARLY = 0  # number of vproj pairs emitted between passA(3) and stage1(1)
VP_PARITY = 0  # vproj copy engine parity (0: DVE on even vp, 1: Act on even)
STA_PARITY = 1  # sta copy engine parity
XD_PARITY = 1  # xdst copy engine parity
OP_PARITY = 0  # out-proj copy engine parity


def build_program(reps: int = 1):
    import contextlib

    nc = bacc.Bacc(
        "TRN2",
        target_bir_lowering=False,
        debug=False,
        enable_asserts=False,
        num_devices=N_CORES,
    )

    # DRAM I/O (per-core data, same names on every core)
    # wpack = [w1T | w2T | w3T | woT | ident(+zero pad)] along the free dim
    qT_d = nc.dram_tensor("qT", [NF, TPC], BF16, kind="ExternalInput").ap()
    vT_d = nc.dram_tensor("vT", [NF, VPAD], BF16, kind="ExternalInput").ap()
    wpack_d = nc.dram_tensor("wpack", [NF, 1148], BF16, kind="ExternalInput").ap()
    maskp_d = nc.dram_tensor("maskp", [128, 2 * H * C], BF16, kind="ExternalInput").ap()
    bidx_d = nc.dram_tensor("bidx", [128, 256], mybir.dt.int16, kind="ExternalInput").ap()
    outT_d = nc.dram_tensor("outT", [NF, TPC], BF16, kind="ExternalOutput").ap()

    with tile.TileContext(nc) as tc:
        with (
            tc.tile_pool(name="inp", bufs=1) as inp,
            tc.tile_pool(name="work", bufs=8) as work,
            tc.tile_pool(name="big_ps", bufs=2, space="PSUM") as big_ps,
            tc.tile_pool(name="sc_ps", bufs=2, space="PSUM") as sc_ps,
            tc.tile_pool(name="tr_ps", bufs=2, space="PSUM") as tr_ps,
            tc.tile_pool(name="x_ps", bufs=2, space="PSUM") as x_ps,
        ):
            # ---- persistent SBUF tensors --------------------------------
            qt_in = inp.tile([128, 2, TPC], BF16, tag="qt_in")
            vt_in = inp.tile([128, 2, VPAD], BF16, tag="vt_in")
            wall = inp.tile([128, 2, 1148], BF16, tag="wall")
            maskp = inp.tile([128, 2 * H * C], BF16, tag="maskp")
            w1t = wall[:, :, 0:256]
            w2t = wall[:, :, 256:508]
            w3t = wall[:, :, 508:764]
            wot = wall[:, :, 764:1020]
            ident = wall[:, 0, 1020:1148]
            mask0 = maskp[:, 0 : H * C]
            mask7 = maskp[:, H * C : 2 * H * C]
            qtr = inp.tile([128, 2, TPC], BF16, tag="qtr")
            vpark = inp.tile([128, 9, NF], BF16, tag="vpark")
            xt = inp.tile([128, 2, TPC], BF16, tag="xt")
            outsb = inp.tile([128, 2, TPC], BF16, tag="outsb")
            bidx = inp.tile([128, 256], mybir.dt.int16, tag="bidx")
            # banded scores built on-chip by local_scatter (one per tile)
            sall = [
                inp.tile([128, SALL_W], BF16, tag=f"sall{i}", name=f"sall{i}")
                for i in range(N_TILES)
            ]
            # S^T per tile: (128 window-rows, 4 heads x 2 chunks x 128 tokens)
            stal = [
                inp.tile([128, H * SW], BF16, tag=f"stal{i}", name=f"stal{i}")
                for i in range(N_TILES)
            ]

            loop_ctx = tc.For_i(0, reps, 1, hint_engines=(mybir.EngineType.PE,)) if reps > 1 else contextlib.nullcontext()
            with loop_ctx:
                # ---- input DMAs, ordered by first use ----------------------
                qT_r = qT_d.rearrange("(c p) t -> p c t", p=128)
                vT_r = vT_d.rearrange("(c p) t -> p c t", p=128)
                wp_r = wpack_d.rearrange("(c p) t -> p c t", p=128)
                if SPLIT_IN_DMA:
                    nc.sync.dma_start(wall[:, :, 0:256], wp_r[:, :, 0:256])
                    nc.sync.dma_start(qt_in[:, :, 0:512], qT_r[:, :, 0:512])
                    nc.sync.dma_start(maskp[:], maskp_d)
                    nc.sync.dma_start(wall[:, :, 256:1148], wp_r[:, :, 256:1148])
                    nc.sync.dma_start(qt_in[:, :, 512:TPC], qT_r[:, :, 512:TPC])
                    nc.sync.dma_start(bidx[:], bidx_d)
                    nc.sync.dma_start(vt_in[:, :, 0:256], vT_r[:, :, 0:256])
                    nc.sync.dma_start(vt_in[:, :, 256:704], vT_r[:, :, 256:704])
                    nc.sync.dma_start(vt_in[:, :, 704:VPAD], vT_r[:, :, 704:VPAD])
                else:
                    nc.sync.dma_start(wall[:], wp_r)
                    nc.sync.dma_start(qt_in[:, :, 0:512], qT_r[:, :, 0:512])
                    nc.sync.dma_start(maskp[:], maskp_d)
                    nc.sync.dma_start(qt_in[:, :, 512:TPC], qT_r[:, :, 512:TPC])
                    nc.sync.dma_start(bidx[:], bidx_d)
                    nc.sync.dma_start(vt_in[:, :, 0:576], vT_r[:, :, 0:576])
                    nc.sync.dma_start(vt_in[:, :, 576:VPAD], vT_r[:, :, 576:VPAD])

                # ---- stage 1 (token range): qTr = relu(w1 @ queryT) --------
                def stage1(lo, hi):
                    w = hi - lo
                    if w <= 256:  # both mc chunks fit one PSUM bank
                        ps = big_ps.tile([128, 2, 256], FP32, tag="big")
                        pviews = [ps[:, mc, 0:w] for mc in range(2)]
                    else:
                        psa = big_ps.tile([128, 512], FP32, tag="big", name="psa")
                        psb = big_ps.tile([128, 512], FP32, tag="big", name="psb")
                        pviews = [psa[:, 0:w], psb[:, 0:w]]
                    for mc in range(2):
                        for kc in range(2):
                            nc.tensor.matmul(
                                pviews[mc],
                                w1t[:, kc, mc * 128 : (mc + 1) * 128],
                                qt_in[:, kc, lo:hi],
                                start=(kc == 0),
                                stop=(kc == 1),
                            )
                        dst = qtr[:, mc, lo:hi]
                        if mc == 0:
                            nc.scalar.activation(
                                dst, pviews[mc], mybir.ActivationFunctionType.Relu
                            )
                        else:
                            nc.vector.tensor_relu(dst, pviews[mc])

                # ---- pass A (per tile pair): scores -> softmax -> scatter --
                def passA_pair(t0, split=False):
                    sc = sc_ps.tile([128, 2, H * C], FP32, tag="sc")
                    def softmax_scatter(u0, nu):
                        scv = sc[:, u0 : u0 + nu, :]
                        expp = work.tile([128, nu, H * C], BF16, tag="expp")
                        nc.scalar.activation(
                            expp[:], scv, mybir.ActivationFunctionType.Exp
                        )
                        den = work.tile([128, nu * H], FP32, tag="den")
                        nc.vector.tensor_reduce(
                            den[:],
                            expp[:].rearrange("p a (h c) -> p (a h) c", h=H),
                            axis=mybir.AxisListType.X,
                            op=mybir.AluOpType.add,
                        )
                        rden = work.tile([128, nu * H], FP32, tag="rden")
                        nc.vector.reciprocal(rden[:], den[:])
                        pn = work.tile([128, nu, H * C], BF16, tag="pn")
                        for u in range(nu):
                            t = t0 + u0 + u
                            rb = AP(rden[:].tensor, rden[:].offset + u * H,
                                    [[nu * H, 128], [1, H], [0, C]])
                            if MUL_ENGINE == "vector" or (MUL_ENGINE == "alternate" and u == 0):
                                mul_eng = nc.vector
                            else:
                                mul_eng = nc.gpsimd
                            mul_eng.tensor_mul(
                                pn[:, u, :].rearrange("p (h c) -> p h c", h=H),
                                expp[:, u, :].rearrange("p (h c) -> p h c", h=H),
                                rb,
                            )
                            # on-chip banded scatter: per token-partition i,
                            #   sall[i, h*SWS+i+k] = pn[i, h*C+k]; rest zeroed
                            nc.gpsimd.local_scatter(
                                sall[t][:], pn[:, u, :], bidx[:, 0 : H * C],
                                channels=128, num_elems=SALL_W, num_idxs=H * C,
                            )
                    for u in range(2):
                        t = t0 + u
                        for kc in range(2):
                            nc.tensor.matmul(
                                sc[:, u, :],
                                qtr[:, kc, t * 128 : (t + 1) * 128],
                                w2t[:, kc, :],
                                start=(kc == 0),
                                stop=(kc == 1),
                            )
                        if t == 0:
                            nc.vector.tensor_add(sc[:, 0, :], sc[:, 0, :], mask0)
                        if t == N_TILES - 1:
                            nc.vector.tensor_add(sc[:, 1, :], sc[:, 1, :], mask7)
                        if split:
                            softmax_scatter(u, 1)
                    if not split:
                        softmax_scatter(0, 2)

                # ---- stage 2: V = value @ w3.T parked at -31 offset --------
                def vproj(vps):
                    for vp in vps:  # pairs of V tiles share one PSUM bank
                        nv = 2 if vp < 4 else 1
                        ps = big_ps.tile([128, 512], FP32, tag="big")
                        for j in range(nv):
                            vt = 2 * vp + j
                            for kc in range(2):
                                nc.tensor.matmul(
                                    ps[:, j * 256 : (j + 1) * 256],
                                    vt_in[:, kc, vt * 128 : (vt + 1) * 128],
                                    w3t[:, kc, :],
                                    start=(kc == 0),
                                    stop=(kc == 1),
                                )
                        if vp % 2 == VP_PARITY:
                            nc.vector.tensor_copy(
                                vpark[:, 2 * vp : 2 * vp + nv, :],
                                ps[:, 0 : nv * 256].rearrange("p (a b) -> p a b", a=nv),
                            )
                        else:
                            nc.scalar.activation(
                                vpark[:, 2 * vp : 2 * vp + nv, :],
                                ps[:, 0 : nv * 256].rearrange("p (a b) -> p a b", a=nv),
                                mybir.ActivationFunctionType.Copy,
                            )

                # ---- pass B (per tile): transpose -> band matmul -> x ------
                def passB(t):
                    st = sall[t]
                    sta = stal[t]
                    if t in DMA_T_TILES:
                        # xbar transposes straight into SBUF (no PSUM copy);
                        # c1 chunks read 128 cols (tail overlaps next head's
                        # band / slack, transposed rows 62+ are never read)
                        for h in range(H):
                            nc.sync.dma_start_transpose(
                                sta[:, h * SW : h * SW + 128],
                                st[:, h * SWS : h * SWS + 128],
                            )
                            nc.sync.dma_start_transpose(
                                sta[:, h * SW + 128 : h * SW + 256],
                                st[:, h * SWS + 128 : h * SWS + 256],
                            )
                    else:
                        trp = tr_ps.tile([128, H * SW], BF16, tag="trp")
                        for h in range(H):
                            nc.tensor.transpose(
                                trp[:, h * SW : h * SW + 128],
                                st[:, h * SWS : h * SWS + 128],
                                ident,
                            )
                            nc.tensor.transpose(
                                trp[0:64, h * SW + 128 : h * SW + 256],
                                st[:, h * SWS + 128 : h * SWS + 192],
                                ident,
                            )
                        # c1 rows 64..127 are stale PSUM junk; band matmuls
                        # only read rows 0..61 of each c1 chunk.
                        if t % 2 == STA_PARITY:
                            nc.vector.tensor_copy(sta[:], trp[:])
                        else:
                            nc.scalar.activation(
                                sta[:], trp[:], mybir.ActivationFunctionType.Copy
                            )
                    # band matmuls: xT_h = V_ext^T @ S^T  (window chunks are
                    # park-tile aligned thanks to the -31 park offset)
                    xps = x_ps.tile([128, 256], FP32, tag="xv")
                    for h in range(H):
                        out_sl = xps[64 * (h % 2) : 64 * (h % 2) + 64,
                                     128 * (h // 2) : 128 * (h // 2) + 128]
                        nc.tensor.matmul(
                            out_sl,
                            vpark[0:128, t, h * DK : (h + 1) * DK],
                            sta[0:128, h * SW : h * SW + 128],
                            start=True,
                            stop=False,
                        )
                        nc.tensor.matmul(
                            out_sl,
                            vpark[0:62, t + 1, h * DK : (h + 1) * DK],
                            sta[0:62, h * SW + 128 : h * SW + 256],
                            start=False,
                            stop=True,
                        )
                    # one copy per tile: (h0,h1 | h2,h3) -> xt feature chunks
                    xdst = AP(
                        xt[:].tensor,
                        xt[:].offset + t * 128,
                        [[2 * TPC, 128], [TPC, 2], [1, 128]],
                    )
                    xsrc = xps[:].rearrange("p (a b) -> p a b", a=2)
                    if t % 2 == XD_PARITY:
                        nc.vector.tensor_copy(xdst, xsrc)
                    else:
                        nc.scalar.activation(
                            xdst, xsrc, mybir.ActivationFunctionType.Copy
                        )

                # ---- out-proj (token range) --------------------------------
                def outproj(lo, hi):
                    w = hi - lo
                    outT_r = outT_d.rearrange("(c p) t -> p c t", p=128)
                    ps = big_ps.tile([128, 2, 256], FP32, tag="big")
                    for mc in range(2):
                        for kc in range(2):
                            nc.tensor.matmul(
                                ps[:, mc, 0:w],
                                wot[:, kc, mc * 128 : (mc + 1) * 128],
                                xt[:, kc, lo:hi],
                                start=(kc == 0),
                                stop=(kc == 1),
                            )
                        osb = outsb[:, mc, lo:hi]
                        if mc == OP_PARITY:
                            nc.vector.tensor_copy(osb, ps[:, mc, 0:w])
                        else:
                            nc.scalar.activation(
                                osb, ps[:, mc, 0:w],
                                mybir.ActivationFunctionType.Copy,
                            )
                        nc.sync.dma_start(outT_r[:, mc, lo:hi], osb)

                # ---- emission order: pass A ASAP, vproj off the A chain ----
                stage1(0, 512)
                passA_pair(0, split=True)
                passA_pair(2)
                vproj(range(VPROJ_EARLY))
                stage1(512, 1024)
                passA_pair(4)
                passA_pair(6)
                vproj(range(VPROJ_EARLY, 5))
                for t in range(N_TILES):
                    passB(t)
                    if t == 7:
                        outproj(768, 1024)
                    elif t % 2 == 1:
                        outproj((t // 2) * 256, (t // 2 + 1) * 256)

    nc.compile()
    return nc


def make_inputs(query, value, w1, w2, w3, w_out):
    """Host-side shard/transpose/cast. Returns per-core in_maps."""
    fq = np.asarray(query, np.float32).reshape(B * T, NF)
    fv = np.asarray(value, np.float32).reshape(B * T, NF)
    wpack = np.zeros((NF, 1148), np.float32)
    wpack[:, 0:256] = np.asarray(w1, np.float32).T
    wpack[:, 256:508] = np.asarray(w2, np.float32).T
    wpack[:, 508:764] = np.asarray(w3, np.float32).T
    wpack[:, 764:1020] = np.asarray(w_out, np.float32).T
    wpack[0:128, 1020:1148] = np.eye(128, dtype=np.float32)
    wpack = wpack.astype(NP_BF16)

    # local_scatter indices: idx[i, h*C + k] = h*SWS + i + k
    bidx = np.zeros((128, 256), np.int16)
    ii = np.arange(128)[:, None, None]
    hh = np.arange(H)[None, :, None]
    kk = np.arange(C)[None, None, :]
    bidx[:, 0 : H * C] = (hh * SWS + ii + kk).reshape(128, H * C)

    in_maps = []
    for c in range(N_CORES):
        t0 = c * TPC
        b = (c * TPC) // T
        b0, b1 = b * T, (b + 1) * T
        qT = np.ascontiguousarray(fq[t0 : t0 + TPC].T).astype(NP_BF16)
        # parked value rows: global tokens [t0-31, t0-31+VPAD), zero outside batch
        vrows = np.zeros((VPAD, NF), np.float32)
        lo = t0 - HALF
        s0, s1 = max(lo, b0), min(lo + VPAD, b1)
        vrows[s0 - lo : s1 - lo] = fv[s0:s1]
        vT = np.ascontiguousarray(vrows.T).astype(NP_BF16)
        # additive band masks for first/last tile (batch edges only)
        maskp = np.zeros((128, 2 * H * C), np.float32)
        k = np.arange(C)
        for i in range(128):
            g = t0 + i
            bad = (g + k - HALF < b0) | (g + k - HALF >= b1)
            maskp[i, : H * C] = np.tile(np.where(bad, -30000.0, 0.0), H)
            g = t0 + (N_TILES - 1) * 128 + i
            bad = (g + k - HALF < b0) | (g + k - HALF >= b1)
            maskp[i, H * C :] = np.tile(np.where(bad, -30000.0, 0.0), H)
        in_maps.append({"qT": qT, "vT": vT, "wpack": wpack,
                        "maskp": maskp.astype(NP_BF16), "bidx": bidx})
    return in_maps


_NC_CACHE = None


def kernel(query, key, value, mask, w1, w2, w3, w_out):
    global _NC_CACHE
    if _NC_CACHE is None:
        _NC_CACHE = build_program()
    nc = _NC_CACHE
    in_maps = make_inputs(query, value, w1, w2, w3, w_out)
    res = bass_utils.run_bass_kernel_spmd(nc, in_maps, core_ids=list(range(N_CORES)))
    outs = []
    for c in range(N_CORES):
        outT = res.results[c]["outT"]  # (256, 1024)
        outs.append(np.ascontiguousarray(outT.T))
    full = np.concatenate(outs, axis=0)  # (8192, 256)
    return full.reshape(B, T, NF).astype(np.float32)


# revision 57
# speedup vs baseline: 1.0140x; 1.0140x over previous
"""Trainium2 Bass kernel for LocalDenseSynthesizerAttention (band C=63, H=4 heads).

Sharding: 8192 tokens (B=2 x T=4096 flattened) split contiguously across 8
cores (1024 tokens each).  Each core runs an identical program on its own
slice; batch-edge band masking and value halo padding are handled host-side
via per-core input data (masks / zero-padded valueT), so the program is
uniform SPMD.

Layouts on device are feature-major ("transposed"): activations live as
(feat, token) so every projection is a plain lhsT.T @ rhs matmul.  The band
scatter runs fully on-chip via the GPSIMD local_scatter instruction
(dst[:]=0; dst[:, idx]=data per partition), turning compact scores
(token, offset) into a banded matrix in SBUF, which is then PE-transposed
into (window_row, token) orientation for the band matmuls.  Copies and
softmax ops are distributed across Act/DVE/Pool to balance engine load.
"""

import numpy as np
import ml_dtypes

import concourse.bass as bass
import concourse.bacc as bacc
import concourse.mybir as mybir
import concourse.tile as tile
from concourse.ap import AP
from concourse import bass_utils

BF16 = mybir.dt.bfloat16
FP32 = mybir.dt.float32
NP_BF16 = ml_dtypes.bfloat16

B, T, NF = 2, 4096, 256
H, C, DK = 4, 63, 64
HALF = (C - 1) // 2  # 31
N_CORES = 8
TPC = (B * T) // N_CORES  # 1024 tokens per core
N_TILES = TPC // 128  # 8
VPAD = 1152  # parked value rows: tokens [-31, 1121) relative to core start
SW = 256  # per-head width of the transposed S^T buffer (2 chunks of 128)
SWS = 192  # per-head stride of the banded score buffer (window rows 0..189+pad)
SALL_W = 768  # banded buffer width (4 heads x 192)
DMA_T_TILES = ()  # tiles whose transposes run on the DMA xbar instead of PE
MUL_ENGINE = "vector"  # engine for the softmax normalize multiply
SPLIT_IN_DMA = True  # split wpack/vt input DMAs by first use
VPROJ_EARLY = 0  # number of vproj pairs emitted between passA(3) and stage1(1)
VP_PARITY = 0  # vproj copy engine parity (0: DVE on even vp, 1: Act on even)
STA_PARITY = 1  # sta copy engine parity
XD_PARITY = 1  # xdst copy engine parity
OP_PARITY = 0  # out-proj copy engine parity


def build_program(reps: int = 1):
    import contextlib

    nc = bacc.Bacc(
        "TRN2",
        target_bir_lowering=False,
        debug=False,
        enable_asserts=False,
        num_devices=N_CORES,
    )

    # DRAM I/O (per-core data, same names on every core)
    # wpack = [w1T | w2T | w3T | woT | ident(+zero pad)] along the free dim
    qT_d = nc.dram_tensor("qT", [NF, TPC], BF16, kind="ExternalInput").ap()
    vT_d = nc.dram_tensor("vT", [NF, VPAD], BF16, kind="ExternalInput").ap()
    wpack_d = nc.dram_tensor("wpack", [NF, 1148], BF16, kind="ExternalInput").ap()
    maskp_d = nc.dram_tensor("maskp", [128, 2 * H * C], BF16, kind="ExternalInput").ap()
    bidx_d = nc.dram_tensor("bidx", [128, 256], mybir.dt.int16, kind="ExternalInput").ap()
    outT_d = nc.dram_tensor("outT", [NF, TPC], BF16, kind="ExternalOutput").ap()

    with tile.TileContext(nc) as tc:
        with (
            tc.tile_pool(name="inp", bufs=1) as inp,
            tc.tile_pool(name="work", bufs=8) as work,
            tc.tile_pool(name="big_ps", bufs=2, space="PSUM") as big_ps,
            tc.tile_pool(name="sc_ps", bufs=2, space="PSUM") as sc_ps,
            tc.tile_pool(name="tr_ps", bufs=2, space="PSUM") as tr_ps,
            tc.tile_pool(name="x_ps", bufs=2, space="PSUM") as x_ps,
        ):
            # ---- persistent SBUF tensors --------------------------------
            qt_in = inp.tile([128, 2, TPC], BF16, tag="qt_in")
            vt_in = inp.tile([128, 2, VPAD], BF16, tag="vt_in")
            wall = inp.tile([128, 2, 1148], BF16, tag="wall")
            maskp = inp.tile([128, 2 * H * C], BF16, tag="maskp")
            w1t = wall[:, :, 0:256]
            w2t = wall[:, :, 256:508]
            w3t = wall[:, :, 508:764]
            wot = wall[:, :, 764:1020]
            ident = wall[:, 0, 1020:1148]
            mask0 = maskp[:, 0 : H * C]
            mask7 = maskp[:, H * C : 2 * H * C]
            qtr = inp.tile([128, 2, TPC], BF16, tag="qtr")
            vpark = inp.tile([128, 9, NF], BF16, tag="vpark")
            xt = inp.tile([128, 2, TPC], BF16, tag="xt")
            outsb = inp.tile([128, 2, TPC], BF16, tag="outsb")
            bidx = inp.tile([128, 256], mybir.dt.int16, tag="bidx")
            # banded scores built on-chip by local_scatter (one per tile)
            sall = [
                inp.tile([128, SALL_W], BF16, tag=f"sall{i}", name=f"sall{i}")
                for i in range(N_TILES)
            ]
            # S^T per tile: (128 window-rows, 4 heads x 2 chunks x 128 tokens)
            stal = [
                inp.tile([128, H * SW], BF16, tag=f"stal{i}", name=f"stal{i}")
                for i in range(N_TILES)
            ]

            loop_ctx = tc.For_i(0, reps, 1, hint_engines=(mybir.EngineType.PE,)) if reps > 1 else contextlib.nullcontext()
            with loop_ctx:
                # ---- input DMAs, ordered by first use ----------------------
                qT_r = qT_d.rearrange("(c p) t -> p c t", p=128)
                vT_r = vT_d.rearrange("(c p) t -> p c t", p=128)
                wp_r = wpack_d.rearrange("(c p) t -> p c t", p=128)
                if SPLIT_IN_DMA:
                    nc.sync.dma_start(wall[:, :, 0:256], wp_r[:, :, 0:256])
                    nc.sync.dma_start(qt_in[:, :, 0:512], qT_r[:, :, 0:512])
                    nc.sync.dma_start(maskp[:], maskp_d)
                    nc.sync.dma_start(wall[:, :, 256:1148], wp_r[:, :, 256:1148])
                    nc.sync.dma_start(qt_in[:, :, 512:TPC], qT_r[:, :, 512:TPC])
                    nc.sync.dma_start(bidx[:], bidx_d)
                    nc.sync.dma_start(vt_in[:, :, 0:256], vT_r[:, :, 0:256])
                    nc.sync.dma_start(vt_in[:, :, 256:704], vT_r[:, :, 256:704])
                    nc.sync.dma_start(vt_in[:, :, 704:VPAD], vT_r[:, :, 704:VPAD])
                else:
                    nc.sync.dma_start(wall[:], wp_r)
                    nc.sync.dma_start(qt_in[:, :, 0:512], qT_r[:, :, 0:512])
                    nc.sync.dma_start(maskp[:], maskp_d)
                    nc.sync.dma_start(qt_in[:, :, 512:TPC], qT_r[:, :, 512:TPC])
                    nc.sync.dma_start(bidx[:], bidx_d)
                    nc.sync.dma_start(vt_in[:, :, 0:576], vT_r[:, :, 0:576])
                    nc.sync.dma_start(vt_in[:, :, 576:VPAD], vT_r[:, :, 576:VPAD])

                # ---- stage 1 (token range): qTr = relu(w1 @ queryT) --------
                def stage1(lo, hi):
                    w = hi - lo
                    if w <= 256:  # both mc chunks fit one PSUM bank
                        ps = big_ps.tile([128, 2, 256], FP32, tag="big")
                        pviews = [ps[:, mc, 0:w] for mc in range(2)]
                    else:
                        psa = big_ps.tile([128, 512], FP32, tag="big", name="psa")
                        psb = big_ps.tile([128, 512], FP32, tag="big", name="psb")
                        pviews = [psa[:, 0:w], psb[:, 0:w]]
                    for mc in range(2):
                        for kc in range(2):
                            nc.tensor.matmul(
                                pviews[mc],
                                w1t[:, kc, mc * 128 : (mc + 1) * 128],
                                qt_in[:, kc, lo:hi],
                                start=(kc == 0),
                                stop=(kc == 1),
                            )
                        dst = qtr[:, mc, lo:hi]
                        if mc == 0:
                            nc.scalar.activation(
                                dst, pviews[mc], mybir.ActivationFunctionType.Relu
                            )
                        else:
                            nc.vector.tensor_relu(dst, pviews[mc])

                # ---- pass A (per tile pair): scores -> softmax -> scatter --
                def passA_pair(t0, split=False):
                    sc = sc_ps.tile([128, 2, H * C], FP32, tag="sc")
                    def softmax_scatter(u0, nu):
                        scv = sc[:, u0 : u0 + nu, :]
                        expp = work.tile([128, nu, H * C], BF16, tag="expp")
                        nc.scalar.activation(
                            expp[:], scv, mybir.ActivationFunctionType.Exp
                        )
                        den = work.tile([128, nu * H], FP32, tag="den")
                        nc.vector.tensor_reduce(
                            den[:],
                            expp[:].rearrange("p a (h c) -> p (a h) c", h=H),
                            axis=mybir.AxisListType.X,
                            op=mybir.AluOpType.add,
                        )
                        rden = work.tile([128, nu * H], FP32, tag="rden")
                        nc.vector.reciprocal(rden[:], den[:])
                        pn = work.tile([128, nu, H * C], BF16, tag="pn")
                        for u in range(nu):
                            t = t0 + u0 + u
                            rb = AP(rden[:].tensor, rden[:].offset + u * H,
                                    [[nu * H, 128], [1, H], [0, C]])
                            if MUL_ENGINE == "vector" or (MUL_ENGINE == "alternate" and u == 0):
                                mul_eng = nc.vector
                            else:
                                mul_eng = nc.gpsimd
                            mul_eng.tensor_mul(
                                pn[:, u, :].rearrange("p (h c) -> p h c", h=H),
                                expp[:, u, :].rearrange("p (h c) -> p h c", h=H),
                                rb,
                            )
                            # on-chip banded scatter: per token-partition i,
                            #   sall[i, h*SWS+i+k] = pn[i, h*C+k]; rest zeroed
                            nc.gpsimd.local_scatter(
                                sall[t][:], pn[:, u, :], bidx[:, 0 : H * C],
                                channels=128, num_elems=SALL_W, num_idxs=H * C,
                            )
                    for u in range(2):
                        t = t0 + u
                        for kc in range(2):
                            nc.tensor.matmul(
                                sc[:, u, :],
                                qtr[:, kc, t * 128 : (t + 1) * 128],
                                w2t[:, kc, :],
                                start=(kc == 0),
                                stop=(kc == 1),
                            )
                        if t == 0:
                            nc.vector.tensor_add(sc[:, 0, :], sc[:, 0, :], mask0)
                        if t == N_TILES - 1:
                            nc.vector.tensor_add(sc[:, 1, :], sc[:, 1, :], mask7)
                        if split:
                            softmax_scatter(u, 1)
                    if not split:
                        softmax_scatter(0, 2)

                # ---- stage 2: V = value @ w3.T parked at -31 offset --------
                def vproj(vps):
                    for vp in vps:  # pairs of V tiles share one PSUM bank
                        nv = 2 if vp < 4 else 1
                        ps = big_ps.tile([128, 512], FP32, tag="big")
                        for j in range(nv):
                            vt = 2 * vp + j
                            for kc in range(2):
                                nc.tensor.matmul(
                                    ps[:, j * 256 : (j + 1) * 256],
                                    vt_in[:, kc, vt * 128 : (vt + 1) * 128],
                                    w3t[:, kc, :],
                                    start=(kc == 0),
                                    stop=(kc == 1),
                                )
                        if vp % 2 == VP_PARITY:
                            nc.vector.tensor_copy(
                                vpark[:, 2 * vp : 2 * vp + nv, :],
                                ps[:, 0 : nv * 256].rearrange("p (a b) -> p a b", a=nv),
                            )
                        else:
                            nc.scalar.activation(
                                vpark[:, 2 * vp : 2 * vp + nv, :],
                                ps[:, 0 : nv * 256].rearrange("p (a b) -> p a b", a=nv),
                                mybir.ActivationFunctionType.Copy,
                            )

                # ---- pass B stage 1 (per tile): transpose + sta copy -------
                def passB_trans(t):
                    st = sall[t]
                    sta = stal[t]
                    if t in DMA_T_TILES:
                        # xbar transposes straight into SBUF (no PSUM copy);
                        # c1 chunks read 128 cols (tail overlaps next head's
                        # band / slack, transposed rows 62+ are never read)
                        for h in range(H):
                            nc.sync.dma_start_transpose(
                                sta[:, h * SW : h * SW + 128],
                                st[:, h * SWS : h * SWS + 128],
                            )
                            nc.sync.dma_start_transpose(
                                sta[:, h * SW + 128 : h * SW + 256],
                                st[:, h * SWS + 128 : h * SWS + 256],
                            )
                    else:
                        trp = tr_ps.tile([128, H * SW], BF16, tag="trp")
                        for h in range(H):
                            nc.tensor.transpose(
                                trp[:, h * SW : h * SW + 128],
                                st[:, h * SWS : h * SWS + 128],
                                ident,
                            )
                            nc.tensor.transpose(
                                trp[0:64, h * SW + 128 : h * SW + 256],
                                st[:, h * SWS + 128 : h * SWS + 192],
                                ident,
                            )
                        # c1 rows 64..127 are stale PSUM junk; band matmuls
                        # only read rows 0..61 of each c1 chunk.
                        if t % 2 == STA_PARITY:
                            nc.vector.tensor_copy(sta[:], trp[:])
                        else:
                            nc.scalar.activation(
                                sta[:], trp[:], mybir.ActivationFunctionType.Copy
                            )
                # ---- pass B stage 2 (per tile): band matmul -> x -----------
                def passB_band(t):
                    sta = stal[t]
                    # band matmuls: xT_h = V_ext^T @ S^T  (window chunks are
                    # park-tile aligned thanks to the -31 park offset)
                    xps = x_ps.tile([128, 256], FP32, tag="xv")
                    for h in range(H):
                        out_sl = xps[64 * (h % 2) : 64 * (h % 2) + 64,
                                     128 * (h // 2) : 128 * (h // 2) + 128]
                        nc.tensor.matmul(
                            out_sl,
                            vpark[0:128, t, h * DK : (h + 1) * DK],
                            sta[0:128, h * SW : h * SW + 128],
                            start=True,
                            stop=False,
                        )
                        nc.tensor.matmul(
                            out_sl,
                            vpark[0:62, t + 1, h * DK : (h + 1) * DK],
                            sta[0:62, h * SW + 128 : h * SW + 256],
                            start=False,
                            stop=True,
                        )
                    # one copy per tile: (h0,h1 | h2,h3) -> xt feature chunks
                    xdst = AP(
                        xt[:].tensor,
                        xt[:].offset + t * 128,
                        [[2 * TPC, 128], [TPC, 2], [1, 128]],
                    )
                    xsrc = xps[:].rearrange("p (a b) -> p a b", a=2)
                    if t % 2 == XD_PARITY:
                        nc.vector.tensor_copy(xdst, xsrc)
                    else:
                        nc.scalar.activation(
                            xdst, xsrc, mybir.ActivationFunctionType.Copy
                        )

                # ---- out-proj (token range) --------------------------------
                def outproj(lo, hi):
                    w = hi - lo
                    outT_r = outT_d.rearrange("(c p) t -> p c t", p=128)
                    ps = big_ps.tile([128, 2, 256], FP32, tag="big")
                    for mc in range(2):
                        for kc in range(2):
                            nc.tensor.matmul(
                                ps[:, mc, 0:w],
                                wot[:, kc, mc * 128 : (mc + 1) * 128],
                                xt[:, kc, lo:hi],
                                start=(kc == 0),
                                stop=(kc == 1),
                            )
                        osb = outsb[:, mc, lo:hi]
                        if mc == OP_PARITY:
                            nc.vector.tensor_copy(osb, ps[:, mc, 0:w])
                        else:
                            nc.scalar.activation(
                                osb, ps[:, mc, 0:w],
                                mybir.ActivationFunctionType.Copy,
                            )
                        nc.sync.dma_start(outT_r[:, mc, lo:hi], osb)

                # ---- emission order: pass A ASAP, vproj off the A chain ----
                stage1(0, 512)
                passA_pair(0, split=True)
                passA_pair(2)
                vproj(range(VPROJ_EARLY))
                stage1(512, 1024)
                passA_pair(4)
                passA_pair(6)
                vproj(range(VPROJ_EARLY, 5))
                for t in range(N_TILES + 1):
                    if t < N_TILES:
                        passB_trans(t)
                    if t > 0:
                        passB_band(t - 1)
                        if t - 1 == 7:
                            outproj(768, 1024)
                        elif (t - 1) % 2 == 1:
                            outproj(((t - 1) // 2) * 256, ((t - 1) // 2 + 1) * 256)

    nc.compile()
    return nc


def make_inputs(query, value, w1, w2, w3, w_out):
    """Host-side shard/transpose/cast. Returns per-core in_maps."""
    fq = np.asarray(query, np.float32).reshape(B * T, NF)
    fv = np.asarray(value, np.float32).reshape(B * T, NF)
    wpack = np.zeros((NF, 1148), np.float32)
    wpack[:, 0:256] = np.asarray(w1, np.float32).T
    wpack[:, 256:508] = np.asarray(w2, np.float32).T
    wpack[:, 508:764] = np.asarray(w3, np.float32).T
    wpack[:, 764:1020] = np.asarray(w_out, np.float32).T
    wpack[0:128, 1020:1148] = np.eye(128, dtype=np.float32)
    wpack = wpack.astype(NP_BF16)

    # local_scatter indices: idx[i, h*C + k] = h*SWS + i + k
    bidx = np.zeros((128, 256), np.int16)
    ii = np.arange(128)[:, None, None]
    hh = np.arange(H)[None, :, None]
    kk = np.arange(C)[None, None, :]
    bidx[:, 0 : H * C] = (hh * SWS + ii + kk).reshape(128, H * C)

    in_maps = []
    for c in range(N_CORES):
        t0 = c * TPC
        b = (c * TPC) // T
        b0, b1 = b * T, (b + 1) * T
        qT = np.ascontiguousarray(fq[t0 : t0 + TPC].T).astype(NP_BF16)
        # parked value rows: global tokens [t0-31, t0-31+VPAD), zero outside batch
        vrows = np.zeros((VPAD, NF), np.float32)
        lo = t0 - HALF
        s0, s1 = max(lo, b0), min(lo + VPAD, b1)
        vrows[s0 - lo : s1 - lo] = fv[s0:s1]
        vT = np.ascontiguousarray(vrows.T).astype(NP_BF16)
        # additive band masks for first/last tile (batch edges only)
        maskp = np.zeros((128, 2 * H * C), np.float32)
        k = np.arange(C)
        for i in range(128):
            g = t0 + i
            bad = (g + k - HALF < b0) | (g + k - HALF >= b1)
            maskp[i, : H * C] = np.tile(np.where(bad, -30000.0, 0.0), H)
            g = t0 + (N_TILES - 1) * 128 + i
            bad = (g + k - HALF < b0) | (g + k - HALF >= b1)
            maskp[i, H * C :] = np.tile(np.where(bad, -30000.0, 0.0), H)
        in_maps.append({"qT": qT, "vT": vT, "wpack": wpack,
                        "maskp": maskp.astype(NP_BF16), "bidx": bidx})
    return in_maps


_NC_CACHE = None


def kernel(query, key, value, mask, w1, w2, w3, w_out):
    global _NC_CACHE
    if _NC_CACHE is None:
        _NC_CACHE = build_program()
    nc = _NC_CACHE
    in_maps = make_inputs(query, value, w1, w2, w3, w_out)
    res = bass_utils.run_bass_kernel_spmd(nc, in_maps, core_ids=list(range(N_CORES)))
    outs = []
    for c in range(N_CORES):
        outT = res.results[c]["outT"]  # (256, 1024)
        outs.append(np.ascontiguousarray(outT.T))
    full = np.concatenate(outs, axis=0)  # (8192, 256)
    return full.reshape(B, T, NF).astype(np.float32)
